# revision 6
# baseline (speedup 1.0000x reference)
"""AMICO ADMM solver on 8 TRN2 NeuronCores.

min_x ||y - A x||^2 + lambda*|x|_1, x >= 0 via ADMM (100 iterations),
data-parallel over voxels (1024 voxels per core).

Shifted-variable restructuring (rho=1, kappa=lambda/rho), carrying
  s   := z - u + kappa = |v - kappa|        (matmul rhs, fp16)
  mb  := u - kappa     = min(v - kappa, 0)  (fp16)
with v = x + u, x = D + W @ s, D = W@AtY - kappa*(W@1):
  psum = W @ s + D          # 8 fp16 matmuls + 4 fp8 DoubleRow injects of
                            #   D = D8a + D8b (compensated fp8, one DR mm
                            #   per psum tile via lhsT = [I; I])
  vb   = psum + mb          # tensor_tensor add, split DVE / GpSimd
  s'   = |vb|               # ScalarE Abs (per row-block, chain-critical)
  mb'  = min(vb, 0)         # DVE tensor_scalar (fp16 4x perf mode)
Final output: x_100 = psum_100 directly.
"""

import os

import numpy as np

M = 256
K = 256
N_VOX = 8192
N_CORES = 8
N_SHARD = N_VOX // N_CORES  # 1024
RHO = 1.0
LAMBDA_REG = 0.1
KAPPA = LAMBDA_REG / RHO
N_ITERS = 100

# Tuning knobs
CHI = int(os.environ.get("KERNEL_CHI", "1024"))  # GpSimd cols of each 1024-col min
D_MODE = os.environ.get("KERNEL_DMODE", "f8c")  # 'f8c' | 'f16'

LAST_RESULTS = None  # BassKernelResults of the most recent run (for test.py)


def _build_graph():
    import concourse.mybir as mybir
    from concourse import bacc
    from concourse.tile import TileContext

    f32 = mybir.dt.float32
    fp16 = mybir.dt.float16
    fp8 = mybir.dt.float8e4
    kap = float(KAPPA)
    alu = mybir.AluOpType

    nc = bacc.Bacc("TRN2", target_bir_lowering=False, debug=False)

    W16_p = nc.declare_dram_parameter("W16", [128, 512], fp16, isOutput=False)
    if D_MODE == "f8c":
        D8_p = nc.declare_dram_parameter("D8", [128, 4, 1024], fp8, isOutput=False)
        I2_p = nc.declare_dram_parameter("I2", [128, 2, 128], fp8, isOutput=False)
    else:
        D16_p = nc.declare_dram_parameter("D16", [128, 2048], fp16, isOutput=False)
        I16_p = nc.declare_dram_parameter("I16", [128, 128], fp16, isOutput=False)
    O_p = nc.declare_dram_parameter("out", [128, 2048], f32, isOutput=True)

    absf = mybir.ActivationFunctionType.Abs
    dr = mybir.MatmulPerfMode.DoubleRow

    with TileContext(nc) as tc:
        with (
            tc.tile_pool(name="static", bufs=1) as statics,
            tc.tile_pool(name="spool", bufs=3) as spool,
            tc.tile_pool(name="vpool", bufs=3) as vpool,
            tc.tile_pool(name="mpool", bufs=3) as mpool,
        ):
            W16_sb = statics.tile([128, 512], fp16, name="W16_sb")
            nc.sync.dma_start(W16_sb[:, :], W16_p[:, :])
            if D_MODE == "f8c":
                D8_sb = statics.tile([128, 4, 1024], fp8, name="D8_sb")
                nc.sync.dma_start(D8_sb[:, :, :], D8_p[:, :, :])
                I2_sb = statics.tile([128, 2, 128], fp8, name="I2_sb")
                nc.sync.dma_start(I2_sb[:, :, :], I2_p[:, :, :])
            else:
                D16_sb = statics.tile([128, 2048], fp16, name="D16_sb")
                nc.sync.dma_start(D16_sb[:, :], D16_p[:, :])
                I16_sb = statics.tile([128, 128], fp16, name="I16_sb")
                nc.sync.dma_start(I16_sb[:, :], I16_p[:, :])
            out_sb = statics.tile([128, 2048], f32, name="out_sb")

            # Warm the ScalarE activation table (Abs) outside the loop.
            warm_in = statics.tile([1, 8], fp16, name="warm_in")
            nc.vector.memset(warm_in[:, :], 0.25)
            warm_sb = statics.tile([1, 8], fp16, name="warm_sb")
            nc.scalar.activation(warm_sb[:, :], warm_in[:, :], absf)

            s_cur = spool.tile([128, 2048], fp16, name="s_new", tag="s")
            nc.vector.memset(s_cur[:, :], kap)
            mb_cur = mpool.tile([128, 2048], fp16, name="mb_new", tag="mb")
            nc.vector.memset(mb_cur[:, :], -kap)

            with tc.tile_pool(name="psum_loop", bufs=4, space="PSUM") as psl:
                for it in range(N_ITERS):
                    last = it == N_ITERS - 1
                    ps = [None, None]
                    for r in (0, 1):
                        ps[r] = psl.tile([128, 1024], f32, name="ps_x", tag="ps")
                    # Inject D into psum (start=True resets the banks).
                    for r in (0, 1):
                        for ch in (0, 1):
                            cs = slice(ch * 512, ch * 512 + 512)
                            if D_MODE == "f8c":
                                nc.tensor.matmul(
                                    ps[r][:, cs],
                                    I2_sb[:, :, :],
                                    D8_sb[:, 2 * r : 2 * r + 2, cs],
                                    start=True,
                                    stop=False,
                                    perf_mode=dr,
                                    skip_group_check=True,
                                )
                            else:
                                nc.tensor.matmul(
                                    ps[r][:, cs],
                                    I16_sb[:, :],
                                    D16_sb[:, r * 1024 + ch * 512 : r * 1024 + ch * 512 + 512],
                                    start=True,
                                    stop=False,
                                    skip_group_check=True,
                                )
                    # psum += W @ s  (weight block (kc, r) reused for both
                    # column halves; kc accumulates).
                    for kc in (0, 1):
                        for r in (0, 1):
                            w0 = kc * 256 + r * 128
                            for ch in (0, 1):
                                cs = slice(ch * 512, ch * 512 + 512)
                                nc.tensor.matmul(
                                    ps[r][:, cs],
                                    W16_sb[:, w0 : w0 + 128],
                                    s_cur[:, kc * 1024 + ch * 512 : kc * 1024 + ch * 512 + 512],
                                    start=False,
                                    stop=(kc == 1),
                                    skip_group_check=True,
                                )

                    if last:
                        for r in (0, 1):
                            rs = slice(r * 1024, r * 1024 + 1024)
                            nc.scalar.copy(out_sb[:, rs], ps[r][:, :])
                            nc.sync.dma_start(O_p[:, rs], out_sb[:, rs])
                        break

                    vb = vpool.tile([128, 2048], fp16, name="vb", tag="vb")
                    sn = spool.tile([128, 2048], fp16, name="s_new", tag="s")
                    mb_new = mpool.tile([128, 2048], fp16, name="mb_new", tag="mb")
                    for r in (0, 1):
                        base = r * 1024
                        # GPSIMD cannot read PSUM: the whole v-add is on DVE.
                        nc.vector.tensor_add(
                            vb[:, base : base + 1024],
                            ps[r][:, :],
                            mb_cur[:, base : base + 1024],
                        )
                        # s'_r = |vb_r| on ScalarE (chain-critical: next
                        # iteration's kc=0 matmuls need the r=0 block first).
                        nc.scalar.activation(
                            sn[:, base : base + 1024], vb[:, base : base + 1024], absf
                        )
                        # mb'_r = min(vb_r, 0): GpSimd cols [0, CHI), DVE rest.
                        if CHI > 0:
                            nc.gpsimd.tensor_scalar_min(
                                mb_new[:, base : base + CHI],
                                vb[:, base : base + CHI],
                                0.0,
                            )
                        if CHI < 1024:
                            nc.vector.tensor_scalar_min(
                                mb_new[:, base + CHI : base + 1024],
                                vb[:, base + CHI : base + 1024],
                                0.0,
                            )
                    s_cur, mb_cur = sn, mb_new

    nc.compile()
    return nc


_GRAPH = None


def kernel(A: np.ndarray, data: np.ndarray) -> np.ndarray:
    global _GRAPH, LAST_RESULTS
    import ml_dtypes
    from concourse.bass_utils import run_bass_kernel_spmd

    F8 = ml_dtypes.float8_e4m3

    A = np.ascontiguousarray(np.asarray(A, dtype=np.float32))
    data = np.ascontiguousarray(np.asarray(data, dtype=np.float32))
    assert A.shape == (M, K) and data.shape == (N_VOX, M)

    A64 = A.astype(np.float64)
    AtA = A64.T @ A64
    W = np.linalg.inv(AtA + RHO * np.eye(K))
    w1 = KAPPA * (W @ np.ones(K))

    # W16[p, kc*256 + c] = W[kc*128+p, c]  (W symmetric)
    W_dev = (
        W.astype(np.float32).reshape(2, 128, K).transpose(1, 0, 2).reshape(128, 2 * K)
    )
    W16_dev = W_dev.astype(np.float16)

    if D_MODE == "f8c":
        # I2[k, j, p] = (p == k) for j in {0,1}
        i2 = np.zeros((128, 2, 128), dtype=F8)
        eye = np.eye(128, dtype=np.float32).astype(F8)
        i2[:, 0, :] = eye
        i2[:, 1, :] = eye
    else:
        i16 = np.eye(128, dtype=np.float16)

    in_maps = []
    for i in range(N_CORES):
        shard = data[i * N_SHARD : (i + 1) * N_SHARD]  # [1024, 256]
        AtY = A64.T @ shard.astype(np.float64).T  # [256, 1024]
        D = (W @ AtY) - w1[:, None]  # [256, 1024] f64
        if D_MODE == "f8c":
            Dr = D.reshape(2, 128, N_SHARD)  # [r, p, n]
            Da = Dr.astype(F8)
            Db = (Dr - Da.astype(np.float64)).astype(F8)
            D8 = np.empty((128, 4, N_SHARD), dtype=F8)
            for r in (0, 1):
                D8[:, 2 * r + 0, :] = Da[r]
                D8[:, 2 * r + 1, :] = Db[r]
            in_maps.append(
                {
                    "W16": W16_dev,
                    "D8": np.ascontiguousarray(D8),
                    "I2": np.ascontiguousarray(i2),
                }
            )
        else:
            # D16[p, r*1024 + n] = D[r*128+p, n]
            D_dev = (
                D.astype(np.float16)
                .reshape(2, 128, N_SHARD)
                .transpose(1, 0, 2)
                .reshape(128, 2 * N_SHARD)
            )
            in_maps.append(
                {
                    "W16": W16_dev,
                    "D16": np.ascontiguousarray(D_dev),
                    "I16": i16,
                }
            )
    if _GRAPH is None:
        _GRAPH = _build_graph()

    trace = bool(int(os.environ.get("KERNEL_TRACE", "0")))
    res = run_bass_kernel_spmd(
        _GRAPH, in_maps, core_ids=list(range(N_CORES)), trace=trace
    )
    LAST_RESULTS = res

    out = np.empty((N_VOX, K), dtype=np.float32)
    for i in range(N_CORES):
        o = res.results[i]["out"]  # [128, 2048]
        X = o.reshape(128, 2, N_SHARD).transpose(1, 0, 2).reshape(K, N_SHARD)
        out[i * N_SHARD : (i + 1) * N_SHARD] = X.T
    return out


# revision 9
# speedup vs baseline: 6.1718x; 6.1718x over previous
"""AMICO ADMM solver on 8 TRN2 NeuronCores.

min_x ||y - A x||^2 + lambda*|x|_1, x >= 0 via ADMM (100 iterations),
data-parallel over voxels (1024 voxels per core).

Shifted-variable restructuring (rho=1, kappa=lambda/rho), carrying
  s   := z - u + kappa = |v - kappa|        (matmul rhs, fp16)
  mb  := u - kappa     = min(v - kappa, 0)  (fp16)
with v = x + u, x = D + W @ s, D = W@AtY - kappa*(W@1):
  psum = W @ s + D          # 8 fp16 matmuls + 4 fp8 DoubleRow injects of
                            #   D = D8a + D8b (compensated fp8, one DR mm
                            #   per psum tile via lhsT = [I; I])
  vb   = psum + mb          # tensor_tensor add, split DVE / GpSimd
  s'   = |vb|               # ScalarE Abs (per row-block, chain-critical)
  mb'  = min(vb, 0)         # DVE tensor_scalar (fp16 4x perf mode)
Final output: x_100 = psum_100 directly.
"""

import os

import numpy as np

M = 256
K = 256
N_VOX = 8192
N_CORES = 8
N_SHARD = N_VOX // N_CORES  # 1024
RHO = 1.0
LAMBDA_REG = 0.1
KAPPA = LAMBDA_REG / RHO
N_ITERS = 100

# Tuning knobs
CHI = int(os.environ.get("KERNEL_CHI", "1024"))  # GpSimd cols of each 1024-col min
D_MODE = os.environ.get("KERNEL_DMODE", "f8c")  # 'f8c' | 'f16'

LAST_RESULTS = None  # BassKernelResults of the most recent run (for test.py)


def _build_graph():
    import concourse.mybir as mybir
    from concourse import bacc
    from concourse.tile import TileContext

    f32 = mybir.dt.float32
    fp16 = mybir.dt.float16
    fp8 = mybir.dt.float8e4
    kap = float(KAPPA)
    alu = mybir.AluOpType

    nc = bacc.Bacc("TRN2", target_bir_lowering=False, debug=False)

    W16_p = nc.declare_dram_parameter("W16", [128, 512], fp16, isOutput=False)
    if D_MODE == "f8c":
        D8_p = nc.declare_dram_parameter("D8", [128, 4, 1024], fp8, isOutput=False)
        I2_p = nc.declare_dram_parameter("I2", [128, 2, 128], fp8, isOutput=False)
    else:
        D16_p = nc.declare_dram_parameter("D16", [128, 2048], fp16, isOutput=False)
        I16_p = nc.declare_dram_parameter("I16", [128, 128], fp16, isOutput=False)
    O_p = nc.declare_dram_parameter("out", [128, 2048], f32, isOutput=True)

    absf = mybir.ActivationFunctionType.Abs
    dr = mybir.MatmulPerfMode.DoubleRow

    with TileContext(nc) as tc:
        with (
            tc.tile_pool(name="static", bufs=1) as statics,
            tc.tile_pool(name="spool", bufs=3) as spool,
            tc.tile_pool(name="vpool", bufs=3) as vpool,
            tc.tile_pool(name="mpool", bufs=3) as mpool,
        ):
            W16_sb = statics.tile([128, 512], fp16, name="W16_sb")
            nc.sync.dma_start(W16_sb[:, :], W16_p[:, :])
            if D_MODE == "f8c":
                D8_sb = statics.tile([128, 4, 1024], fp8, name="D8_sb")
                nc.sync.dma_start(D8_sb[:, :, :], D8_p[:, :, :])
                I2_sb = statics.tile([128, 2, 128], fp8, name="I2_sb")
                nc.sync.dma_start(I2_sb[:, :, :], I2_p[:, :, :])
            else:
                D16_sb = statics.tile([128, 2048], fp16, name="D16_sb")
                nc.sync.dma_start(D16_sb[:, :], D16_p[:, :])
                I16_sb = statics.tile([128, 128], fp16, name="I16_sb")
                nc.sync.dma_start(I16_sb[:, :], I16_p[:, :])
            out_sb = statics.tile([128, 2048], f32, name="out_sb")

            # Warm the ScalarE activation table (Abs) outside the loop.
            warm_in = statics.tile([1, 8], fp16, name="warm_in")
            nc.vector.memset(warm_in[:, :], 0.25)
            warm_sb = statics.tile([1, 8], fp16, name="warm_sb")
            nc.scalar.activation(warm_sb[:, :], warm_in[:, :], absf)

            s_cur = spool.tile([128, 2048], fp16, name="s_new", tag="s")
            nc.vector.memset(s_cur[:, :], kap)
            vb_cur = vpool.tile([128, 2048], fp16, name="vb", tag="vb")
            nc.vector.memset(vb_cur[:, :], -kap)

            with tc.tile_pool(name="psum_loop", bufs=4, space="PSUM") as psl:
                for it in range(N_ITERS):
                    last = it == N_ITERS - 1
                    ps = [None, None]
                    for r in (0, 1):
                        ps[r] = psl.tile([128, 1024], f32, name="ps_x", tag="ps")
                    # Per r-half: inject D (start=True resets banks), then
                    # accumulate W @ s.  Completing ps[0] after 6 matmuls
                    # (instead of 10) gives its elementwise chain a head
                    # start that overlaps PE's r=1 half.
                    for r in (0, 1):
                        for ch in (0, 1):
                            cs = slice(ch * 512, ch * 512 + 512)
                            if D_MODE == "f8c":
                                nc.tensor.matmul(
                                    ps[r][:, cs],
                                    I2_sb[:, :, :],
                                    D8_sb[:, 2 * r : 2 * r + 2, cs],
                                    start=True,
                                    stop=False,
                                    perf_mode=dr,
                                    skip_group_check=True,
                                )
                            else:
                                nc.tensor.matmul(
                                    ps[r][:, cs],
                                    I16_sb[:, :],
                                    D16_sb[:, r * 1024 + ch * 512 : r * 1024 + ch * 512 + 512],
                                    start=True,
                                    stop=False,
                                    skip_group_check=True,
                                )
                        for kc in (0, 1):
                            w0 = kc * 256 + r * 128
                            for ch in (0, 1):
                                cs = slice(ch * 512, ch * 512 + 512)
                                nc.tensor.matmul(
                                    ps[r][:, cs],
                                    W16_sb[:, w0 : w0 + 128],
                                    s_cur[:, kc * 1024 + ch * 512 : kc * 1024 + ch * 512 + 512],
                                    start=False,
                                    stop=(kc == 1),
                                    skip_group_check=True,
                                )

                    if last:
                        for r in (0, 1):
                            rs = slice(r * 1024, r * 1024 + 1024)
                            nc.scalar.copy(out_sb[:, rs], ps[r][:, :])
                            nc.sync.dma_start(O_p[:, rs], out_sb[:, rs])
                        break

                    vb_new = vpool.tile([128, 2048], fp16, name="vb", tag="vb")
                    sn = spool.tile([128, 2048], fp16, name="s_new", tag="s")
                    for r in (0, 1):
                        base = r * 1024
                        # Fused: vb' = min(vb, 0) + psum  (one 1x DVE pass;
                        # the min rides free, u/m never materializes).
                        nc.vector.scalar_tensor_tensor(
                            vb_new[:, base : base + 1024],
                            vb_cur[:, base : base + 1024],
                            0.0,
                            ps[r][:, :],
                            alu.min,
                            alu.add,
                        )
                        # s'_r = |vb'_r| on ScalarE (chain-critical: next
                        # iteration's kc=0 matmuls need the r=0 block first).
                        nc.scalar.activation(
                            sn[:, base : base + 1024],
                            vb_new[:, base : base + 1024],
                            absf,
                        )
                    s_cur, vb_cur = sn, vb_new

    nc.compile()
    return nc


_GRAPH = None


def kernel(A: np.ndarray, data: np.ndarray) -> np.ndarray:
    global _GRAPH, LAST_RESULTS
    import ml_dtypes
    from concourse.bass_utils import run_bass_kernel_spmd

    F8 = ml_dtypes.float8_e4m3

    A = np.ascontiguousarray(np.asarray(A, dtype=np.float32))
    data = np.ascontiguousarray(np.asarray(data, dtype=np.float32))
    assert A.shape == (M, K) and data.shape == (N_VOX, M)

    A64 = A.astype(np.float64)
    AtA = A64.T @ A64
    W = np.linalg.inv(AtA + RHO * np.eye(K))
    w1 = KAPPA * (W @ np.ones(K))

    # W16[p, kc*256 + c] = W[kc*128+p, c]  (W symmetric)
    W_dev = (
        W.astype(np.float32).reshape(2, 128, K).transpose(1, 0, 2).reshape(128, 2 * K)
    )
    W16_dev = W_dev.astype(np.float16)

    if D_MODE == "f8c":
        # I2[k, j, p] = (p == k) for j in {0,1}
        i2 = np.zeros((128, 2, 128), dtype=F8)
        eye = np.eye(128, dtype=np.float32).astype(F8)
        i2[:, 0, :] = eye
        i2[:, 1, :] = eye
    else:
        i16 = np.eye(128, dtype=np.float16)

    in_maps = []
    for i in range(N_CORES):
        shard = data[i * N_SHARD : (i + 1) * N_SHARD]  # [1024, 256]
        AtY = A64.T @ shard.astype(np.float64).T  # [256, 1024]
        D = (W @ AtY) - w1[:, None]  # [256, 1024] f64
        if D_MODE == "f8c":
            Dr = D.reshape(2, 128, N_SHARD)  # [r, p, n]
            Da = Dr.astype(F8)
            Db = (Dr - Da.astype(np.float64)).astype(F8)
            D8 = np.empty((128, 4, N_SHARD), dtype=F8)
            for r in (0, 1):
                D8[:, 2 * r + 0, :] = Da[r]
                D8[:, 2 * r + 1, :] = Db[r]
            in_maps.append(
                {
                    "W16": W16_dev,
                    "D8": np.ascontiguousarray(D8),
                    "I2": np.ascontiguousarray(i2),
                }
            )
        else:
            # D16[p, r*1024 + n] = D[r*128+p, n]
            D_dev = (
                D.astype(np.float16)
                .reshape(2, 128, N_SHARD)
                .transpose(1, 0, 2)
                .reshape(128, 2 * N_SHARD)
            )
            in_maps.append(
                {
                    "W16": W16_dev,
                    "D16": np.ascontiguousarray(D_dev),
                    "I16": i16,
                }
            )
    if _GRAPH is None:
        _GRAPH = _build_graph()

    trace = bool(int(os.environ.get("KERNEL_TRACE", "0")))
    res = run_bass_kernel_spmd(
        _GRAPH, in_maps, core_ids=list(range(N_CORES)), trace=trace
    )
    LAST_RESULTS = res

    out = np.empty((N_VOX, K), dtype=np.float32)
    for i in range(N_CORES):
        o = res.results[i]["out"]  # [128, 2048]
        X = o.reshape(128, 2, N_SHARD).transpose(1, 0, 2).reshape(K, N_SHARD)
        out[i * N_SHARD : (i + 1) * N_SHARD] = X.T
    return out


# revision 10
# speedup vs baseline: 9.4836x; 1.5366x over previous
"""AMICO ADMM solver on 8 TRN2 NeuronCores.

min_x ||y - A x||^2 + lambda*|x|_1, x >= 0 via ADMM (100 iterations),
data-parallel over voxels (1024 voxels per core).

Shifted-variable restructuring (rho=1, kappa=lambda/rho), carrying
  s   := z - u + kappa = |v - kappa|        (matmul rhs, fp16)
  mb  := u - kappa     = min(v - kappa, 0)  (fp16)
with v = x + u, x = D + W @ s, D = W@AtY - kappa*(W@1):
  psum = W @ s + D          # 8 fp16 matmuls + 4 fp8 DoubleRow injects of
                            #   D = D8a + D8b (compensated fp8, one DR mm
                            #   per psum tile via lhsT = [I; I])
  vb   = psum + mb          # tensor_tensor add, split DVE / GpSimd
  s'   = |vb|               # ScalarE Abs (per row-block, chain-critical)
  mb'  = min(vb, 0)         # DVE tensor_scalar (fp16 4x perf mode)
Final output: x_100 = psum_100 directly.
"""

import os

import numpy as np

M = 256
K = 256
N_VOX = 8192
N_CORES = 8
N_SHARD = N_VOX // N_CORES  # 1024
RHO = 1.0
LAMBDA_REG = 0.1
KAPPA = LAMBDA_REG / RHO
N_ITERS = 100

# Tuning knobs
CHI = int(os.environ.get("KERNEL_CHI", "1024"))  # GpSimd cols of each 1024-col min
D_MODE = os.environ.get("KERNEL_DMODE", "f8c")  # 'f8c' | 'f16'

LAST_RESULTS = None  # BassKernelResults of the most recent run (for test.py)


def _build_graph():
    import concourse.mybir as mybir
    from concourse import bacc
    from concourse.tile import TileContext

    f32 = mybir.dt.float32
    fp16 = mybir.dt.float16
    fp8 = mybir.dt.float8e4
    kap = float(KAPPA)
    alu = mybir.AluOpType

    nc = bacc.Bacc("TRN2", target_bir_lowering=False, debug=False)

    W16_p = nc.declare_dram_parameter("W16", [128, 512], fp16, isOutput=False)
    if D_MODE == "f8c":
        D8_p = nc.declare_dram_parameter("D8", [128, 4, 1024], fp8, isOutput=False)
        I2_p = nc.declare_dram_parameter("I2", [128, 2, 128], fp8, isOutput=False)
    else:
        D16_p = nc.declare_dram_parameter("D16", [128, 2048], fp16, isOutput=False)
        I16_p = nc.declare_dram_parameter("I16", [128, 128], fp16, isOutput=False)
    O_p = nc.declare_dram_parameter("out", [128, 2048], f32, isOutput=True)

    absf = mybir.ActivationFunctionType.Abs
    dr = mybir.MatmulPerfMode.DoubleRow

    with TileContext(nc) as tc:
        with (
            tc.tile_pool(name="static", bufs=1) as statics,
            tc.tile_pool(name="spool", bufs=3) as spool,
            tc.tile_pool(name="vpool", bufs=3) as vpool,
            tc.tile_pool(name="mpool", bufs=3) as mpool,
        ):
            W16_sb = statics.tile([128, 512], fp16, name="W16_sb")
            nc.sync.dma_start(W16_sb[:, :], W16_p[:, :])
            if D_MODE == "f8c":
                D8_sb = statics.tile([128, 4, 1024], fp8, name="D8_sb")
                nc.sync.dma_start(D8_sb[:, :, :], D8_p[:, :, :])
                I2_sb = statics.tile([128, 2, 128], fp8, name="I2_sb")
                nc.sync.dma_start(I2_sb[:, :, :], I2_p[:, :, :])
            else:
                D16_sb = statics.tile([128, 2048], fp16, name="D16_sb")
                nc.sync.dma_start(D16_sb[:, :], D16_p[:, :])
                I16_sb = statics.tile([128, 128], fp16, name="I16_sb")
                nc.sync.dma_start(I16_sb[:, :], I16_p[:, :])
            out_sb = statics.tile([128, 2048], f32, name="out_sb")

            # Warm the ScalarE activation table (Abs) outside the loop.
            warm_in = statics.tile([1, 8], fp16, name="warm_in")
            nc.vector.memset(warm_in[:, :], 0.25)
            warm_sb = statics.tile([1, 8], fp16, name="warm_sb")
            nc.scalar.activation(warm_sb[:, :], warm_in[:, :], absf)

            s_cur = spool.tile([128, 2048], fp16, name="s_new", tag="s")
            nc.vector.memset(s_cur[:, :], kap)
            vb_cur = vpool.tile([128, 2048], fp16, name="vb", tag="vb")
            nc.vector.memset(vb_cur[:, :], -kap)

            with tc.tile_pool(name="psum_loop", bufs=4, space="PSUM") as psl:
                for it in range(N_ITERS):
                    last = it == N_ITERS - 1
                    ps = [None, None]
                    for r in (0, 1):
                        ps[r] = psl.tile([128, 1024], f32, name="ps_x", tag="ps")
                    if not last:
                        vb_new = vpool.tile([128, 2048], fp16, name="vb", tag="vb")
                        sn = spool.tile([128, 2048], fp16, name="s_new", tag="s")
                    # Quarter-sliced chains: each psum quarter (r, ch) is
                    # finished after 3 matmuls (inject + kc0 + kc1), then its
                    # 512-wide stt+abs run while PE works the other quarters.
                    for r in (0, 1):
                        for ch in (0, 1):
                            cs = slice(ch * 512, ch * 512 + 512)
                            q0 = r * 1024 + ch * 512
                            qs = slice(q0, q0 + 512)
                            if D_MODE == "f8c":
                                nc.tensor.matmul(
                                    ps[r][:, cs],
                                    I2_sb[:, :, :],
                                    D8_sb[:, 2 * r : 2 * r + 2, cs],
                                    start=True,
                                    stop=False,
                                    perf_mode=dr,
                                    skip_group_check=True,
                                )
                            else:
                                nc.tensor.matmul(
                                    ps[r][:, cs],
                                    I16_sb[:, :],
                                    D16_sb[:, qs],
                                    start=True,
                                    stop=False,
                                    skip_group_check=True,
                                )
                            for kc in (0, 1):
                                w0 = kc * 256 + r * 128
                                nc.tensor.matmul(
                                    ps[r][:, cs],
                                    W16_sb[:, w0 : w0 + 128],
                                    s_cur[:, kc * 1024 + ch * 512 : kc * 1024 + ch * 512 + 512],
                                    start=False,
                                    stop=(kc == 1),
                                    skip_group_check=True,
                                )
                            if last:
                                nc.scalar.copy(out_sb[:, qs], ps[r][:, cs])
                                nc.sync.dma_start(O_p[:, qs], out_sb[:, qs])
                                continue
                            # vb' = min(vb, 0) + psum  (fused 1x DVE pass;
                            # u/m never materializes).
                            nc.vector.scalar_tensor_tensor(
                                vb_new[:, qs],
                                vb_cur[:, qs],
                                0.0,
                                ps[r][:, cs],
                                alu.min,
                                alu.add,
                            )
                            # s' quarter = |vb'| on ScalarE.
                            nc.scalar.activation(
                                sn[:, qs], vb_new[:, qs], absf
                            )
                    if last:
                        break
                    s_cur, vb_cur = sn, vb_new

    nc.compile()
    return nc


_GRAPH = None


def kernel(A: np.ndarray, data: np.ndarray) -> np.ndarray:
    global _GRAPH, LAST_RESULTS
    import ml_dtypes
    from concourse.bass_utils import run_bass_kernel_spmd

    F8 = ml_dtypes.float8_e4m3

    A = np.ascontiguousarray(np.asarray(A, dtype=np.float32))
    data = np.ascontiguousarray(np.asarray(data, dtype=np.float32))
    assert A.shape == (M, K) and data.shape == (N_VOX, M)

    A64 = A.astype(np.float64)
    AtA = A64.T @ A64
    W = np.linalg.inv(AtA + RHO * np.eye(K))
    w1 = KAPPA * (W @ np.ones(K))

    # W16[p, kc*256 + c] = W[kc*128+p, c]  (W symmetric)
    W_dev = (
        W.astype(np.float32).reshape(2, 128, K).transpose(1, 0, 2).reshape(128, 2 * K)
    )
    W16_dev = W_dev.astype(np.float16)

    if D_MODE == "f8c":
        # I2[k, j, p] = (p == k) for j in {0,1}
        i2 = np.zeros((128, 2, 128), dtype=F8)
        eye = np.eye(128, dtype=np.float32).astype(F8)
        i2[:, 0, :] = eye
        i2[:, 1, :] = eye
    else:
        i16 = np.eye(128, dtype=np.float16)

    in_maps = []
    for i in range(N_CORES):
        shard = data[i * N_SHARD : (i + 1) * N_SHARD]  # [1024, 256]
        AtY = A64.T @ shard.astype(np.float64).T  # [256, 1024]
        D = (W @ AtY) - w1[:, None]  # [256, 1024] f64
        if D_MODE == "f8c":
            Dr = D.reshape(2, 128, N_SHARD)  # [r, p, n]
            Da = Dr.astype(F8)
            Db = (Dr - Da.astype(np.float64)).astype(F8)
            D8 = np.empty((128, 4, N_SHARD), dtype=F8)
            for r in (0, 1):
                D8[:, 2 * r + 0, :] = Da[r]
                D8[:, 2 * r + 1, :] = Db[r]
            in_maps.append(
                {
                    "W16": W16_dev,
                    "D8": np.ascontiguousarray(D8),
                    "I2": np.ascontiguousarray(i2),
                }
            )
        else:
            # D16[p, r*1024 + n] = D[r*128+p, n]
            D_dev = (
                D.astype(np.float16)
                .reshape(2, 128, N_SHARD)
                .transpose(1, 0, 2)
                .reshape(128, 2 * N_SHARD)
            )
            in_maps.append(
                {
                    "W16": W16_dev,
                    "D16": np.ascontiguousarray(D_dev),
                    "I16": i16,
                }
            )
    if _GRAPH is None:
        _GRAPH = _build_graph()

    trace = bool(int(os.environ.get("KERNEL_TRACE", "0")))
    res = run_bass_kernel_spmd(
        _GRAPH, in_maps, core_ids=list(range(N_CORES)), trace=trace
    )
    LAST_RESULTS = res

    out = np.empty((N_VOX, K), dtype=np.float32)
    for i in range(N_CORES):
        o = res.results[i]["out"]  # [128, 2048]
        X = o.reshape(128, 2, N_SHARD).transpose(1, 0, 2).reshape(K, N_SHARD)
        out[i * N_SHARD : (i + 1) * N_SHARD] = X.T
    return out
